# revision 7
# baseline (speedup 1.0000x reference)
"""TRN2 Bass kernel for nn_HCSMoEQwen3MoeSparseMoeBlock (8-core, balanced).

Routing is computed on the host (numpy, f64) as part of input sharding:
for each token the top-8 expert probabilities are merged per group into
w[t, g]; only (token, group) pairs with w > 0 are real work (~10.5k of
16.4k here).  That work is balanced across the 8 cores in 128-token
chunks: each core runs NCHUNK chunks split into NSEG=3 segments; each
segment is bound to one (group -> dominant expert) whose gu/dn weights
are streamed into a double-buffered SBUF slot while the previous
segment computes.  Device does, per chunk:
  M1   h = x @ guT    (bf16, xT-chunk stationary, gu moving N=512)
  SwiGLU (scalar silu + DVE mult, host-interleaved [256 gate|256 up])
  PE transpose act -> actT (bf16, 1 cyc/row)
  M2   y = act @ dnT  (bf16, actT stationary, dn moving N=512)
  y *= w[token]  (per-partition scalar), DMA out (f32)
Host scatter-adds the per-slot rows into the full [2048, 2048] output.
"""
import math
from collections import Counter
from itertools import product as _iproduct

import ml_dtypes
import numpy as np

import concourse.bass as bass
import concourse.mybir as mybir
import concourse.tile as tile
from concourse import bacc
from concourse.bass_utils import run_bass_kernel_spmd
from concourse.masks import make_identity

T = 2048
H = 2048
I2 = 1536
I = 768
E = 32
G = 8
TOP_K = 8
KO = H // 128
JO = I // 128
TCH = 128
HB = 512
NSEG = 3

F32 = mybir.dt.float32
BF16 = mybir.dt.bfloat16
AX = mybir.AxisListType.X
OP = mybir.AluOpType
ACTF = mybir.ActivationFunctionType
BF16NP = ml_dtypes.bfloat16

_CACHED_NC = {}


def _build(nchunk, seglens):
    key = (nchunk, tuple(seglens))
    if key in _CACHED_NC:
        return _CACHED_NC[key]
    nc = bacc.Bacc("TRN2", target_bir_lowering=False, debug=False, num_devices=G)

    xT_d = nc.dram_tensor("xT", [H, nchunk * TCH], BF16, kind="ExternalInput")
    gu_d = [nc.dram_tensor(f"gu{s}", [H, I2], BF16, kind="ExternalInput")
            for s in range(NSEG)]
    dn_d = [nc.dram_tensor(f"dn{s}", [I, H], BF16, kind="ExternalInput")
            for s in range(NSEG)]
    wtok_d = nc.dram_tensor("wtok", [TCH, nchunk], F32, kind="ExternalInput")
    y_d = nc.dram_tensor("y", [nchunk * TCH, H], BF16, kind="ExternalOutput")

    xT_ap = xT_d.ap().rearrange("(ko p) t -> p ko t", p=128)
    gu_ap = [t.ap().rearrange("(ko p) o -> p ko o", p=128) for t in gu_d]
    dn_ap = [t.ap().rearrange("(jo p) h -> p jo h", p=128) for t in dn_d]

    with tile.TileContext(nc) as tc:
        with (
            tc.tile_pool(name="const", bufs=1) as cpool,
            tc.tile_pool(name="wgu", bufs=2) as gupool,
            tc.tile_pool(name="wdn", bufs=2) as dnpool,
            tc.tile_pool(name="xin", bufs=3) as xpool,
            tc.tile_pool(name="acts", bufs=2) as apool,
            tc.tile_pool(name="yout", bufs=8) as ypool,
            tc.tile_pool(name="ph", bufs=3, space="PSUM") as pph,
            tc.tile_pool(name="pt", bufs=2, space="PSUM") as ppt,
            tc.tile_pool(name="py", bufs=2, space="PSUM") as ppy,
        ):
            identity = cpool.tile([128, 128], BF16, tag="identity")
            make_identity(nc, identity)
            w_sb = cpool.tile([TCH, nchunk], F32, tag="wtok")
            nc.sync.dma_start(w_sb[:], wtok_d.ap())

            gu_tiles = {}
            dn_tiles = {}
            xtiles = {}

            def load_x(ci):
                t = xpool.tile([128, KO, TCH], BF16, tag="xT_c",
                               name=f"xT_c{ci}")
                nc.sync.dma_start(t[:], xT_ap[:, :, ci * TCH:(ci + 1) * TCH])
                xtiles[ci] = t

            def alloc_seg(s):
                g = gupool.tile([128, KO, I2], BF16, tag="gu", name=f"gu{s}")
                d = dnpool.tile([128, JO, H], BF16, tag="dn", name=f"dn{s}")
                gu_tiles[s] = g
                dn_tiles[s] = d
                return g, d

            def seg_load_pieces(s):
                """DMA thunks loading segment s's weights in ~0.5MB pieces
                (col-blocks of gu in M1 consumption order, then j-rows of
                dn) so x/y DMAs behind them in the queue never stall long."""
                g, d = alloc_seg(s)
                th = []
                for c0 in range(0, I2, 128):
                    th.append(lambda c0=c0: nc.sync.dma_start(
                        g[:, :, c0:c0 + 128],
                        gu_ap[s][:, :, c0:c0 + 128]))
                for j in range(JO):
                    th.append(lambda j=j: nc.sync.dma_start(
                        d[:, j], dn_ap[s][:, j]))
                return th

            # ---- head: seg0 pieces interleaved with x prefetch ----
            load_x(0)
            p0 = seg_load_pieces(0)
            for t in p0[:4]:      # gu cols 0:512 (M1 b0)
                t()
            load_x(1)
            for t in p0[4:12]:    # gu cols 512:1536 (b1, b2)
                t()
            load_x(2)
            for t in p0[12:]:     # dn rows
                t()
            # seg1/seg2 pieces dribbled out across preceding chunks
            pending = seg_load_pieces(1)
            p2_alloc = False

            ci = 0
            for s in range(NSEG):
                gu_sb = gu_tiles[s]
                dn_sb = dn_tiles[s]
                for cc in range(seglens[s]):
                    if ci + 3 < nchunk:
                        load_x(ci + 3)
                    if s == 1 and not p2_alloc:
                        pending = pending + seg_load_pieces(2)
                        p2_alloc = True
                    # issue enough pieces per chunk to finish one segment's
                    # 18 pieces over ~3.5 chunks
                    for _ in range(6):
                        if pending:
                            pending.pop(0)()
                    xT_c = xtiles.pop(ci)

                    # ---- M1 + SwiGLU (b-major, one PSUM bank each) ----
                    act_sb = apool.tile([128, I], BF16, tag="act",
                                        name=f"act{ci}")
                    for b in range(3):
                        h_ps = pph.tile([128, HB], F32, tag="h_ps",
                                        name=f"h{ci}_{b}")
                        for k in range(KO):
                            nc.tensor.matmul(
                                h_ps[:], xT_c[:, k],
                                gu_sb[:, k, b * HB:(b + 1) * HB],
                                start=(k == 0), stop=(k == KO - 1),
                            )
                        silu_sb = apool.tile([128, 256], F32, tag="silu",
                                             name=f"silu{ci}_{b}")
                        nc.scalar.activation(silu_sb[:], h_ps[:, :256],
                                             ACTF.Silu)
                        nc.vector.tensor_tensor(
                            act_sb[:, 256 * b:256 * (b + 1)], silu_sb[:],
                            h_ps[:, 256:], OP.mult,
                        )

                    # ---- transpose act -> actT (bf16, 1 cyc/row) ----
                    actT_sb = apool.tile([128, JO, TCH], BF16, tag="actT",
                                         name=f"actT{ci}")
                    for j in range(JO):
                        tp = ppt.tile([128, TCH], BF16, tag="tp",
                                      name=f"tp{ci}_{j}")
                        nc.tensor.transpose(
                            tp[:], act_sb[:, j * 128:(j + 1) * 128], identity)
                        nc.vector.tensor_copy(actT_sb[:, j], tp[:])

                    # ---- M2 + scale + store ----
                    for hb in range(H // HB):
                        y_ps = ppy.tile([128, HB], F32, tag="y_ps",
                                        name=f"y{ci}_{hb}")
                        for j in range(JO):
                            nc.tensor.matmul(
                                y_ps[:], actT_sb[:, j],
                                dn_sb[:, j, hb * HB:(hb + 1) * HB],
                                start=(j == 0), stop=(j == JO - 1),
                            )
                        y_sb = ypool.tile([128, HB], BF16, tag="y_sb",
                                          name=f"ysb{ci}_{hb}")
                        nc.vector.tensor_scalar(
                            y_sb[:], y_ps[:], w_sb[:, ci:ci + 1], None,
                            OP.mult,
                        )
                        nc.sync.dma_start(
                            y_d.ap()[ci * TCH:(ci + 1) * TCH,
                                     hb * HB:(hb + 1) * HB], y_sb[:],
                        )
                    ci += 1
    nc.compile()
    _CACHED_NC[key] = nc
    return nc


_GATEUP_PERM = np.concatenate(
    [np.r_[256 * b:256 * b + 256, 768 + 256 * b:768 + 256 * b + 256]
     for b in range(3)]
)


def _pack(chunks):
    """Assign per-group chunk counts to 8 cores x NSEG fixed-length
    segments.  Returns (nchunk, seglens, per_seg) where per_seg[s] is the
    length-8 list of group ids (-1 = dummy) for segment s across cores."""
    total = sum(chunks)
    lo = max(NSEG, math.ceil(total / 8)) if total else NSEG
    for nchunk in range(lo, lo + 6):
        base, rem = divmod(nchunk, NSEG)
        seglens = [base + 1] * rem + [base] * (NSEG - rem)
        capc = Counter(seglens)
        vals = sorted(capc, reverse=True)
        avail = {v: 8 * capc[v] for v in vals}
        order = sorted(range(G), key=lambda g: -chunks[g])
        assign = {}

        def dfs(gi):
            if gi == len(order):
                return True
            g = order[gi]
            need = chunks[g]
            if need == 0:
                assign[g] = Counter()
                return dfs(gi + 1)
            combos = []
            for ks in _iproduct(*[range(avail[v] + 1) for v in vals]):
                tot = sum(k * v for k, v in zip(ks, vals))
                if tot >= need and tot - need < min(
                        v for k, v in zip(ks, vals) if k):
                    combos.append((tot - need, sum(ks), ks))
            combos.sort()
            for _, _, ks in combos:
                ok = all(avail[v] >= k for k, v in zip(ks, vals))
                if not ok:
                    continue
                for k, v in zip(ks, vals):
                    avail[v] -= k
                assign[g] = Counter(
                    {v: k for k, v in zip(ks, vals) if k})
                if dfs(gi + 1):
                    return True
                for k, v in zip(ks, vals):
                    avail[v] += k
            return False

        if dfs(0):
            seg_entries = {v: [] for v in vals}
            for g in range(G):
                for v, k in assign.get(g, Counter()).items():
                    seg_entries[v].extend([g] * k)
            per_seg = []
            offs = {v: 0 for v in vals}
            for L in seglens:
                lst = seg_entries[L][offs[L]:offs[L] + 8]
                offs[L] += 8
                lst = lst + [-1] * (8 - len(lst))
                per_seg.append(lst)
            return nchunk, seglens, per_seg
    raise RuntimeError("segment packing failed")


def _route(hidden_states, gate_weight, merge_groups):
    """Host router: returns w [T, G] f64 (combined weight per token/group)."""
    x = np.asarray(hidden_states, np.float64).reshape(-1, H)
    gw = np.asarray(gate_weight, np.float64)
    mg = np.asarray(merge_groups).astype(np.int64)
    logits = x @ gw.T
    m = logits.max(axis=1, keepdims=True)
    p = np.exp(logits - m)
    p /= p.sum(axis=1, keepdims=True)
    top8 = np.argpartition(-p, TOP_K - 1, axis=1)[:, :TOP_K]
    tv = np.take_along_axis(p, top8, 1)
    tv /= tv.sum(axis=1, keepdims=True)
    w = np.zeros((x.shape[0], G), np.float64)
    np.add.at(w, (np.arange(x.shape[0])[:, None], mg[top8]), tv)
    return w


def prepare(hidden_states, gate_weight, gate_up_proj, down_proj,
            merge_groups, dominant_experts):
    w = _route(hidden_states, gate_weight, merge_groups)
    de = np.asarray(dominant_experts).astype(np.int64)
    ids = [np.nonzero(w[:, g] > 0)[0] for g in range(G)]
    chunks = [-(-len(i) // TCH) if len(i) else 0 for i in ids]
    nchunk, seglens, per_seg = _pack(chunks)

    x32 = np.asarray(hidden_states, np.float32).reshape(-1, H)
    gup = np.asarray(gate_up_proj, np.float32)
    dnp_ = np.asarray(down_proj, np.float32)

    # per-expert weight tensors (bf16), computed once per unique expert
    guT_cache = {}
    dnT_cache = {}
    for g in range(G):
        e = int(de[g])
        if e not in guT_cache:
            guT_cache[e] = np.ascontiguousarray(
                gup[e].T[:, _GATEUP_PERM]).astype(BF16NP)
            dnT_cache[e] = np.ascontiguousarray(dnp_[e].T).astype(BF16NP)
    gu_zero = np.zeros((H, I2), BF16NP)
    dn_zero = np.zeros((I, H), BF16NP)

    # distribute each group's tokens over its slots in (seg, core) order
    consumed = [0] * G
    slots = []  # records: (core, seg, chunk_start, n_real, token_ids)
    tok_full = [np.zeros(nchunk * TCH, np.int64) for _ in range(8)]
    w_full = [np.zeros(nchunk * TCH, np.float32) for _ in range(8)]
    seg_start = [sum(seglens[:s]) for s in range(NSEG)]
    core_seg_group = [[-1] * NSEG for _ in range(8)]
    for s in range(NSEG):
        for c in range(8):
            g = per_seg[s][c]
            core_seg_group[c][s] = g
            if g < 0:
                continue
            cap = seglens[s] * TCH
            take = min(cap, len(ids[g]) - consumed[g])
            if take <= 0:
                continue
            tk = ids[g][consumed[g]:consumed[g] + take]
            consumed[g] += take
            off = seg_start[s] * TCH
            tok_full[c][off:off + take] = tk
            w_full[c][off:off + take] = w[tk, g].astype(np.float32)
            slots.append((c, off, take, tk))
    for g in range(G):
        assert consumed[g] == len(ids[g]), "token assignment incomplete"

    in_maps = []
    for c in range(8):
        xT = np.ascontiguousarray(x32[tok_full[c]].T).astype(BF16NP)
        wmat = np.ascontiguousarray(
            w_full[c].reshape(nchunk, TCH).T)
        im = {"xT": xT, "wtok": wmat}
        for s in range(NSEG):
            g = core_seg_group[c][s]
            if g < 0:
                im[f"gu{s}"] = gu_zero
                im[f"dn{s}"] = dn_zero
            else:
                e = int(de[g])
                im[f"gu{s}"] = guT_cache[e]
                im[f"dn{s}"] = dnT_cache[e]
        in_maps.append(im)
    return nchunk, seglens, in_maps, slots


def kernel(hidden_states, gate_weight, gate_up_proj, down_proj,
           merge_groups, dominant_experts):
    nchunk, seglens, in_maps, slots = prepare(
        hidden_states, gate_weight, gate_up_proj, down_proj,
        merge_groups, dominant_experts)
    nc = _build(nchunk, seglens)
    res = run_bass_kernel_spmd(nc, in_maps, core_ids=list(range(8)),
                               trace=False)
    out = np.zeros((T, H), np.float64)
    for c, off, take, tk in slots:
        out[tk] += res.results[c]["y"][off:off + take].astype(np.float64)
    return out.astype(np.float32).reshape(1, T, H)


# revision 9
# speedup vs baseline: 1.1620x; 1.1620x over previous
"""TRN2 Bass kernel for nn_HCSMoEQwen3MoeSparseMoeBlock (8-core, balanced).

Routing is computed on the host (numpy, f64) as part of input sharding:
for each token the top-8 expert probabilities are merged per group into
w[t, g]; only (token, group) pairs with w > 0 are real work (~10.5k of
16.4k here).  That work is balanced across the 8 cores in 128-token
chunks: each core runs NCHUNK chunks split into NSEG=3 segments; each
segment is bound to one (group -> dominant expert) whose gu/dn weights
are streamed into a double-buffered SBUF slot while the previous
segment computes.  Device does, per chunk:
  M1   h = x @ guT    (bf16, xT-chunk stationary, gu moving N=512)
  SwiGLU (scalar silu + DVE mult, host-interleaved [256 gate|256 up])
  PE transpose act -> actT (bf16, 1 cyc/row)
  M2   y = act @ dnT  (bf16, actT stationary, dn moving N=512)
  y *= w[token]  (per-partition scalar), DMA out (f32)
Host scatter-adds the per-slot rows into the full [2048, 2048] output.
"""
import math
from collections import Counter
from itertools import product as _iproduct

import ml_dtypes
import numpy as np

import concourse.bass as bass
import concourse.mybir as mybir
import concourse.tile as tile
from concourse import bacc
from concourse.bass_utils import run_bass_kernel_spmd
from concourse.masks import make_identity

T = 2048
H = 2048
I2 = 1536
I = 768
E = 32
G = 8
TOP_K = 8
KO = H // 128
JO = I // 128
TCH = 128
HB = 512
NSEG = 3

F32 = mybir.dt.float32
BF16 = mybir.dt.bfloat16
AX = mybir.AxisListType.X
OP = mybir.AluOpType
ACTF = mybir.ActivationFunctionType
BF16NP = ml_dtypes.bfloat16

_CACHED_NC = {}


def _build(nchunk, seglens):
    key = (nchunk, tuple(seglens))
    if key in _CACHED_NC:
        return _CACHED_NC[key]
    nc = bacc.Bacc("TRN2", target_bir_lowering=False, debug=False, num_devices=G)

    xT_d = nc.dram_tensor("xT", [H, nchunk * TCH], BF16, kind="ExternalInput")
    gu_d = [nc.dram_tensor(f"gu{s}", [H, I2], BF16, kind="ExternalInput")
            for s in range(NSEG)]
    dn_d = [nc.dram_tensor(f"dn{s}", [I, H], BF16, kind="ExternalInput")
            for s in range(NSEG)]
    wtok_d = nc.dram_tensor("wtok", [TCH, nchunk], F32, kind="ExternalInput")
    y_d = nc.dram_tensor("y", [nchunk * TCH, H], BF16, kind="ExternalOutput")

    xT_ap = xT_d.ap().rearrange("(ko p) t -> p ko t", p=128)
    gu_ap = [t.ap().rearrange("(ko p) o -> p ko o", p=128) for t in gu_d]
    dn_ap = [t.ap().rearrange("(jo p) h -> p jo h", p=128) for t in dn_d]

    with tile.TileContext(nc) as tc:
        with (
            tc.tile_pool(name="const", bufs=1) as cpool,
            tc.tile_pool(name="wgu", bufs=2) as gupool,
            tc.tile_pool(name="wdn", bufs=2) as dnpool,
            tc.tile_pool(name="xin", bufs=3) as xpool,
            tc.tile_pool(name="acts", bufs=2) as apool,
            tc.tile_pool(name="yout", bufs=8) as ypool,
            tc.tile_pool(name="ph", bufs=3, space="PSUM") as pph,
            tc.tile_pool(name="pt", bufs=2, space="PSUM") as ppt,
            tc.tile_pool(name="py", bufs=2, space="PSUM") as ppy,
        ):
            identity = cpool.tile([128, 128], BF16, tag="identity")
            make_identity(nc, identity)
            w_sb = cpool.tile([TCH, nchunk], F32, tag="wtok")
            nc.sync.dma_start(w_sb[:], wtok_d.ap())

            gu_tiles = {}
            dn_tiles = {}
            xtiles = {}

            def load_x(ci):
                t = xpool.tile([128, KO, TCH], BF16, tag="xT_c",
                               name=f"xT_c{ci}")
                nc.sync.dma_start(t[:], xT_ap[:, :, ci * TCH:(ci + 1) * TCH])
                xtiles[ci] = t

            def alloc_seg(s):
                g = gupool.tile([128, KO, I2], BF16, tag="gu", name=f"gu{s}")
                d = dnpool.tile([128, JO, H], BF16, tag="dn", name=f"dn{s}")
                gu_tiles[s] = g
                dn_tiles[s] = d
                return g, d

            def seg_load_pieces(s, split_b0=False):
                """DMA thunks loading segment s's weights in ~1.6-2MB pieces
                with wide (1-4KB) per-partition lines: gu by 512-col b-blocks
                (M1 consumption order), dn by j-row triples."""
                g, d = alloc_seg(s)
                th = []
                if split_b0:
                    # k-granular first block so chunk 0's M1 starts sooner
                    for k0 in range(0, KO, 4):
                        th.append(lambda k0=k0: nc.sync.dma_start(
                            g[:, k0:k0 + 4, 0:HB],
                            gu_ap[s][:, k0:k0 + 4, 0:HB]))
                else:
                    th.append(lambda: nc.sync.dma_start(
                        g[:, :, 0:HB], gu_ap[s][:, :, 0:HB]))
                for b in range(1, 3):
                    th.append(lambda b=b: nc.sync.dma_start(
                        g[:, :, b * HB:(b + 1) * HB],
                        gu_ap[s][:, :, b * HB:(b + 1) * HB]))
                for j0 in range(0, JO, 3):
                    th.append(lambda j0=j0: nc.sync.dma_start(
                        d[:, j0:j0 + 3], dn_ap[s][:, j0:j0 + 3]))
                return th

            # ---- head: seg0 pieces interleaved with x prefetch ----
            load_x(0)
            p0 = seg_load_pieces(0, split_b0=True)
            for t in p0[:4]:      # gu cols 0:512, k-granular (M1 b0)
                t()
            load_x(1)
            for t in p0[4:6]:     # gu b1, b2
                t()
            load_x(2)
            for t in p0[6:]:      # dn rows
                t()
            # seg1/seg2 pieces dribbled out across preceding chunks
            pending = seg_load_pieces(1)
            p2_alloc = False

            ci = 0
            for s in range(NSEG):
                gu_sb = gu_tiles[s]
                dn_sb = dn_tiles[s]
                for cc in range(seglens[s]):
                    if ci + 3 < nchunk:
                        load_x(ci + 3)
                    if s == 1 and not p2_alloc:
                        pending = pending + seg_load_pieces(2)
                        p2_alloc = True
                    # issue enough pieces per chunk to finish one segment's
                    # 5 pieces over ~3 chunks
                    for _ in range(2):
                        if pending:
                            pending.pop(0)()
                    xT_c = xtiles.pop(ci)

                    # ---- M1 + SwiGLU (b-major, one PSUM bank each) ----
                    act_sb = apool.tile([128, I], BF16, tag="act",
                                        name=f"act{ci}")
                    for b in range(3):
                        h_ps = pph.tile([128, HB], F32, tag="h_ps",
                                        name=f"h{ci}_{b}")
                        for k in range(KO):
                            nc.tensor.matmul(
                                h_ps[:], xT_c[:, k],
                                gu_sb[:, k, b * HB:(b + 1) * HB],
                                start=(k == 0), stop=(k == KO - 1),
                            )
                        silu_sb = apool.tile([128, 256], F32, tag="silu",
                                             name=f"silu{ci}_{b}")
                        nc.scalar.activation(silu_sb[:], h_ps[:, :256],
                                             ACTF.Silu)
                        nc.vector.tensor_tensor(
                            act_sb[:, 256 * b:256 * (b + 1)], silu_sb[:],
                            h_ps[:, 256:], OP.mult,
                        )

                    # ---- transpose act -> actT (bf16, 1 cyc/row) ----
                    actT_sb = apool.tile([128, JO, TCH], BF16, tag="actT",
                                         name=f"actT{ci}")
                    for j in range(JO):
                        tp = ppt.tile([128, TCH], BF16, tag="tp",
                                      name=f"tp{ci}_{j}")
                        nc.tensor.transpose(
                            tp[:], act_sb[:, j * 128:(j + 1) * 128], identity)
                        nc.vector.tensor_copy(actT_sb[:, j], tp[:])

                    # ---- M2 + scale + store ----
                    for hb in range(H // HB):
                        y_ps = ppy.tile([128, HB], F32, tag="y_ps",
                                        name=f"y{ci}_{hb}")
                        for j in range(JO):
                            nc.tensor.matmul(
                                y_ps[:], actT_sb[:, j],
                                dn_sb[:, j, hb * HB:(hb + 1) * HB],
                                start=(j == 0), stop=(j == JO - 1),
                            )
                        y_sb = ypool.tile([128, HB], BF16, tag="y_sb",
                                          name=f"ysb{ci}_{hb}")
                        nc.vector.tensor_scalar(
                            y_sb[:], y_ps[:], w_sb[:, ci:ci + 1], None,
                            OP.mult,
                        )
                        nc.sync.dma_start(
                            y_d.ap()[ci * TCH:(ci + 1) * TCH,
                                     hb * HB:(hb + 1) * HB], y_sb[:],
                        )
                    ci += 1
    nc.compile()
    _CACHED_NC[key] = nc
    return nc


_GATEUP_PERM = np.concatenate(
    [np.r_[256 * b:256 * b + 256, 768 + 256 * b:768 + 256 * b + 256]
     for b in range(3)]
)


def _pack(chunks):
    """Assign per-group chunk counts to 8 cores x NSEG fixed-length
    segments.  Returns (nchunk, seglens, per_seg) where per_seg[s] is the
    length-8 list of group ids (-1 = dummy) for segment s across cores."""
    total = sum(chunks)
    lo = max(NSEG, math.ceil(total / 8)) if total else NSEG
    for nchunk in range(lo, lo + 6):
        base, rem = divmod(nchunk, NSEG)
        seglens = [base + 1] * rem + [base] * (NSEG - rem)
        capc = Counter(seglens)
        vals = sorted(capc, reverse=True)
        avail = {v: 8 * capc[v] for v in vals}
        order = sorted(range(G), key=lambda g: -chunks[g])
        assign = {}

        def dfs(gi):
            if gi == len(order):
                return True
            g = order[gi]
            need = chunks[g]
            if need == 0:
                assign[g] = Counter()
                return dfs(gi + 1)
            combos = []
            for ks in _iproduct(*[range(avail[v] + 1) for v in vals]):
                tot = sum(k * v for k, v in zip(ks, vals))
                if tot >= need and tot - need < min(
                        v for k, v in zip(ks, vals) if k):
                    combos.append((tot - need, sum(ks), ks))
            combos.sort()
            for _, _, ks in combos:
                ok = all(avail[v] >= k for k, v in zip(ks, vals))
                if not ok:
                    continue
                for k, v in zip(ks, vals):
                    avail[v] -= k
                assign[g] = Counter(
                    {v: k for k, v in zip(ks, vals) if k})
                if dfs(gi + 1):
                    return True
                for k, v in zip(ks, vals):
                    avail[v] += k
            return False

        if dfs(0):
            seg_entries = {v: [] for v in vals}
            for g in range(G):
                for v, k in assign.get(g, Counter()).items():
                    seg_entries[v].extend([g] * k)
            per_seg = []
            offs = {v: 0 for v in vals}
            for L in seglens:
                lst = seg_entries[L][offs[L]:offs[L] + 8]
                offs[L] += 8
                lst = lst + [-1] * (8 - len(lst))
                per_seg.append(lst)
            return nchunk, seglens, per_seg
    raise RuntimeError("segment packing failed")


def _route(hidden_states, gate_weight, merge_groups):
    """Host router: returns w [T, G] f64 (combined weight per token/group)."""
    x = np.asarray(hidden_states, np.float64).reshape(-1, H)
    gw = np.asarray(gate_weight, np.float64)
    mg = np.asarray(merge_groups).astype(np.int64)
    logits = x @ gw.T
    m = logits.max(axis=1, keepdims=True)
    p = np.exp(logits - m)
    p /= p.sum(axis=1, keepdims=True)
    top8 = np.argpartition(-p, TOP_K - 1, axis=1)[:, :TOP_K]
    tv = np.take_along_axis(p, top8, 1)
    tv /= tv.sum(axis=1, keepdims=True)
    w = np.zeros((x.shape[0], G), np.float64)
    np.add.at(w, (np.arange(x.shape[0])[:, None], mg[top8]), tv)
    return w


def prepare(hidden_states, gate_weight, gate_up_proj, down_proj,
            merge_groups, dominant_experts):
    w = _route(hidden_states, gate_weight, merge_groups)
    de = np.asarray(dominant_experts).astype(np.int64)
    ids = [np.nonzero(w[:, g] > 0)[0] for g in range(G)]
    chunks = [-(-len(i) // TCH) if len(i) else 0 for i in ids]
    nchunk, seglens, per_seg = _pack(chunks)

    x32 = np.asarray(hidden_states, np.float32).reshape(-1, H)
    gup = np.asarray(gate_up_proj, np.float32)
    dnp_ = np.asarray(down_proj, np.float32)

    # per-expert weight tensors (bf16), computed once per unique expert
    guT_cache = {}
    dnT_cache = {}
    for g in range(G):
        e = int(de[g])
        if e not in guT_cache:
            guT_cache[e] = np.ascontiguousarray(
                gup[e].T[:, _GATEUP_PERM]).astype(BF16NP)
            dnT_cache[e] = np.ascontiguousarray(dnp_[e].T).astype(BF16NP)
    gu_zero = np.zeros((H, I2), BF16NP)
    dn_zero = np.zeros((I, H), BF16NP)

    # distribute each group's tokens over its slots in (seg, core) order
    consumed = [0] * G
    slots = []  # records: (core, seg, chunk_start, n_real, token_ids)
    tok_full = [np.zeros(nchunk * TCH, np.int64) for _ in range(8)]
    w_full = [np.zeros(nchunk * TCH, np.float32) for _ in range(8)]
    seg_start = [sum(seglens[:s]) for s in range(NSEG)]
    core_seg_group = [[-1] * NSEG for _ in range(8)]
    for s in range(NSEG):
        for c in range(8):
            g = per_seg[s][c]
            core_seg_group[c][s] = g
            if g < 0:
                continue
            cap = seglens[s] * TCH
            take = min(cap, len(ids[g]) - consumed[g])
            if take <= 0:
                continue
            tk = ids[g][consumed[g]:consumed[g] + take]
            consumed[g] += take
            off = seg_start[s] * TCH
            tok_full[c][off:off + take] = tk
            w_full[c][off:off + take] = w[tk, g].astype(np.float32)
            slots.append((c, off, take, tk))
    for g in range(G):
        assert consumed[g] == len(ids[g]), "token assignment incomplete"

    in_maps = []
    for c in range(8):
        xT = np.ascontiguousarray(x32[tok_full[c]].T).astype(BF16NP)
        wmat = np.ascontiguousarray(
            w_full[c].reshape(nchunk, TCH).T)
        im = {"xT": xT, "wtok": wmat}
        for s in range(NSEG):
            g = core_seg_group[c][s]
            if g < 0:
                im[f"gu{s}"] = gu_zero
                im[f"dn{s}"] = dn_zero
            else:
                e = int(de[g])
                im[f"gu{s}"] = guT_cache[e]
                im[f"dn{s}"] = dnT_cache[e]
        in_maps.append(im)
    return nchunk, seglens, in_maps, slots


def kernel(hidden_states, gate_weight, gate_up_proj, down_proj,
           merge_groups, dominant_experts):
    nchunk, seglens, in_maps, slots = prepare(
        hidden_states, gate_weight, gate_up_proj, down_proj,
        merge_groups, dominant_experts)
    nc = _build(nchunk, seglens)
    res = run_bass_kernel_spmd(nc, in_maps, core_ids=list(range(8)),
                               trace=False)
    out = np.zeros((T, H), np.float64)
    for c, off, take, tk in slots:
        out[tk] += res.results[c]["y"][off:off + take].astype(np.float64)
    return out.astype(np.float32).reshape(1, T, H)


# revision 13
# speedup vs baseline: 1.2027x; 1.0350x over previous
"""TRN2 Bass kernel for nn_HCSMoEQwen3MoeSparseMoeBlock (8-core, balanced).

Routing is computed on the host (numpy, f64) as part of input sharding:
for each token the top-8 expert probabilities are merged per group into
w[t, g]; only (token, group) pairs with w > 0 are real work (~10.5k of
16.4k here).  That work is balanced across the 8 cores in 128-token
chunks: each core runs NCHUNK chunks split into NSEG=3 segments; each
segment is bound to one (group -> dominant expert) whose gu/dn weights
are streamed into a double-buffered SBUF slot while the previous
segment computes.  Device does, per chunk:
  M1   h = x @ guT    (bf16, xT-chunk stationary, gu moving N=512)
  SwiGLU (scalar silu + DVE mult, host-interleaved [256 gate|256 up])
  PE transpose act -> actT (bf16, 1 cyc/row)
  M2   y = act @ dnT  (bf16, actT stationary, dn moving N=512)
  y *= w[token]  (per-partition scalar), DMA out (f32)
Host scatter-adds the per-slot rows into the full [2048, 2048] output.
"""
import math
from collections import Counter
from itertools import product as _iproduct

import ml_dtypes
import numpy as np

import concourse.bass as bass
import concourse.mybir as mybir
import concourse.tile as tile
from concourse import bacc
from concourse.bass_utils import run_bass_kernel_spmd
from concourse.masks import make_identity

T = 2048
H = 2048
I2 = 1536
I = 768
E = 32
G = 8
TOP_K = 8
KO = H // 128
JO = I // 128
TCH = 128
HB = 512
NSEG = 3

F32 = mybir.dt.float32
BF16 = mybir.dt.bfloat16
AX = mybir.AxisListType.X
OP = mybir.AluOpType
ACTF = mybir.ActivationFunctionType
BF16NP = ml_dtypes.bfloat16

_CACHED_NC = {}


def _build(nchunk, seglens):
    key = (nchunk, tuple(seglens))
    if key in _CACHED_NC:
        return _CACHED_NC[key]
    nc = bacc.Bacc("TRN2", target_bir_lowering=False, debug=False, num_devices=G)

    xT_d = nc.dram_tensor("xT", [H, nchunk * TCH], BF16, kind="ExternalInput")
    gu_d = [nc.dram_tensor(f"gu{s}", [H, I2], BF16, kind="ExternalInput")
            for s in range(NSEG)]
    dn_d = [nc.dram_tensor(f"dn{s}", [I, H], BF16, kind="ExternalInput")
            for s in range(NSEG)]
    wtok_d = nc.dram_tensor("wtok", [TCH, nchunk], F32, kind="ExternalInput")
    y_d = nc.dram_tensor("y", [nchunk * TCH, H], BF16, kind="ExternalOutput")

    xT_ap = xT_d.ap().rearrange("(ko p) t -> p ko t", p=128)
    gu_ap = [t.ap().rearrange("(ko p) o -> p ko o", p=128) for t in gu_d]
    dn_ap = [t.ap().rearrange("(jo p) h -> p jo h", p=128) for t in dn_d]

    seg_start = [sum(seglens[:s]) for s in range(NSEG)]
    with tile.TileContext(nc) as tc:
        with (
            tc.tile_pool(name="const", bufs=1) as cpool,
            tc.tile_pool(name="wgu", bufs=2) as gupool,
            tc.tile_pool(name="wdn", bufs=2) as dnpool,
            tc.tile_pool(name="xin", bufs=2) as xpool,
            tc.tile_pool(name="acts", bufs=2) as apool,
            tc.tile_pool(name="silu", bufs=2) as spool,
            tc.tile_pool(name="yout", bufs=8) as ypool,
            tc.tile_pool(name="phg", bufs=2, space="PSUM") as phg,
            tc.tile_pool(name="phu", bufs=2, space="PSUM") as phu,
            tc.tile_pool(name="py", bufs=2, space="PSUM") as ppy,
        ):
            w_sb = cpool.tile([TCH, nchunk], F32, tag="wtok")
            nc.sync.dma_start(w_sb[:], wtok_d.ap())

            gu_tiles = {}
            dn_tiles = {}
            xtiles = {}

            def load_x_piece(s, cc):
                """Load one 128-token slice of segment s's x tile."""
                if s not in xtiles:
                    xtiles[s] = xpool.tile([128, KO, 512], BF16, tag="xs",
                                           name=f"xs{s}")
                ci = seg_start[s] + cc
                nc.sync.dma_start(
                    xtiles[s][:, :, cc * TCH:(cc + 1) * TCH],
                    xT_ap[:, :, ci * TCH:(ci + 1) * TCH])

            def alloc_seg(s):
                g = gupool.tile([128, KO, I2], BF16, tag="gu", name=f"gu{s}")
                d = dnpool.tile([128, JO, H], BF16, tag="dn", name=f"dn{s}")
                gu_tiles[s] = g
                dn_tiles[s] = d
                return g, d

            def seg_load_pieces(s, split_b0=False):
                """DMA thunks loading segment s's weights in ~1.6-2MB pieces
                with wide (1-4KB) per-partition lines: gu by 512-col b-blocks
                (M1 consumption order), dn by j-row triples."""
                g, d = alloc_seg(s)
                th = []
                if split_b0:
                    # k-granular first block so chunk 0's M1 starts sooner
                    for k0 in range(0, KO, 4):
                        th.append(lambda k0=k0: nc.sync.dma_start(
                            g[:, k0:k0 + 4, 0:HB],
                            gu_ap[s][:, k0:k0 + 4, 0:HB]))
                else:
                    th.append(lambda: nc.sync.dma_start(
                        g[:, :, 0:HB], gu_ap[s][:, :, 0:HB]))
                for b in range(1, 3):
                    th.append(lambda b=b: nc.sync.dma_start(
                        g[:, :, b * HB:(b + 1) * HB],
                        gu_ap[s][:, :, b * HB:(b + 1) * HB]))
                for j0 in range(0, JO, 3):
                    th.append(lambda j0=j0: nc.sync.dma_start(
                        d[:, j0:j0 + 3], dn_ap[s][:, j0:j0 + 3]))
                return th

            # ---- head: seg0 pieces interleaved with x prefetch ----
            for cc in range(seglens[0]):
                load_x_piece(0, cc)
            p0 = seg_load_pieces(0, split_b0=True)
            for t in p0:
                t()
            # seg1/seg2 pieces + next-seg x dribbled across the stream
            pending = [lambda cc=cc: load_x_piece(1, cc)
                       for cc in range(seglens[1])]
            pending += seg_load_pieces(1)
            p2_alloc = False

            for s in range(NSEG):
                L = seglens[s]
                NT = L * TCH
                gu_sb = gu_tiles[s]
                dn_sb = dn_tiles[s]
                xs = xtiles.pop(s)
                if s == 1 and not p2_alloc:
                    pending = (
                        [lambda cc=cc: load_x_piece(2, cc)
                         for cc in range(seglens[2])]
                        + pending + seg_load_pieces(2))
                    p2_alloc = True

                # ---- M1 + SwiGLU, whole segment, o-pair major ----
                # gu host-interleaved: col block 2j = gate_j, 2j+1 = up_j
                act = apool.tile([128, JO, 512], BF16, tag="act",
                                 name=f"act{s}")
                for j in range(JO):
                    gps = phg.tile([128, NT], F32, tag="hg",
                                   name=f"hg{s}_{j}")
                    ups = phu.tile([128, NT], F32, tag="hu",
                                   name=f"hu{s}_{j}")
                    for k in range(KO):
                        nc.tensor.matmul(
                            gps[:], gu_sb[:, k, 256 * j:256 * j + 128],
                            xs[:, k, :NT],
                            start=(k == 0), stop=(k == KO - 1),
                        )
                    for k in range(KO):
                        nc.tensor.matmul(
                            ups[:], gu_sb[:, k, 256 * j + 128:256 * j + 256],
                            xs[:, k, :NT],
                            start=(k == 0), stop=(k == KO - 1),
                        )
                    sl = spool.tile([128, 512], F32, tag="silu",
                                    name=f"sl{s}_{j}")
                    nc.scalar.activation(sl[:, :NT], gps[:], ACTF.Silu)
                    nc.vector.tensor_tensor(act[:, j, :NT], sl[:, :NT],
                                            ups[:], OP.mult)
                    for _ in range(1):
                        if pending:
                            pending.pop(0)()

                # ---- M2 + scale + store, per 128-token chunk ----
                for cc in range(L):
                    ci = seg_start[s] + cc
                    for hb in range(H // HB):
                        y_ps = ppy.tile([128, HB], F32, tag="y_ps",
                                        name=f"y{ci}_{hb}")
                        for j in range(JO):
                            nc.tensor.matmul(
                                y_ps[:],
                                act[:, j, cc * TCH:(cc + 1) * TCH],
                                dn_sb[:, j, hb * HB:(hb + 1) * HB],
                                start=(j == 0), stop=(j == JO - 1),
                            )
                        y_sb = ypool.tile([128, HB], BF16, tag="y_sb",
                                          name=f"ysb{ci}_{hb}")
                        nc.vector.tensor_scalar(
                            y_sb[:], y_ps[:], w_sb[:, ci:ci + 1], None,
                            OP.mult,
                        )
                        nc.sync.dma_start(
                            y_d.ap()[ci * TCH:(ci + 1) * TCH,
                                     hb * HB:(hb + 1) * HB], y_sb[:],
                        )
                    for _ in range(2):
                        if pending:
                            pending.pop(0)()
    nc.compile()
    _CACHED_NC[key] = nc
    return nc


_GATEUP_PERM = np.concatenate(
    [np.r_[128 * j:128 * j + 128, 768 + 128 * j:768 + 128 * j + 128]
     for j in range(JO)]
)


def _pack(chunks):
    """Assign per-group chunk counts to 8 cores x NSEG fixed-length
    segments.  Returns (nchunk, seglens, per_seg) where per_seg[s] is the
    length-8 list of group ids (-1 = dummy) for segment s across cores."""
    total = sum(chunks)
    lo = max(NSEG, math.ceil(total / 8)) if total else NSEG
    for nchunk in range(lo, lo + 6):
        base, rem = divmod(nchunk, NSEG)
        seglens = [base + 1] * rem + [base] * (NSEG - rem)
        capc = Counter(seglens)
        vals = sorted(capc, reverse=True)
        avail = {v: 8 * capc[v] for v in vals}
        order = sorted(range(G), key=lambda g: -chunks[g])
        assign = {}

        def dfs(gi):
            if gi == len(order):
                return True
            g = order[gi]
            need = chunks[g]
            if need == 0:
                assign[g] = Counter()
                return dfs(gi + 1)
            combos = []
            for ks in _iproduct(*[range(avail[v] + 1) for v in vals]):
                tot = sum(k * v for k, v in zip(ks, vals))
                if tot >= need and tot - need < min(
                        v for k, v in zip(ks, vals) if k):
                    combos.append((tot - need, sum(ks), ks))
            combos.sort()
            for _, _, ks in combos:
                ok = all(avail[v] >= k for k, v in zip(ks, vals))
                if not ok:
                    continue
                for k, v in zip(ks, vals):
                    avail[v] -= k
                assign[g] = Counter(
                    {v: k for k, v in zip(ks, vals) if k})
                if dfs(gi + 1):
                    return True
                for k, v in zip(ks, vals):
                    avail[v] += k
            return False

        if dfs(0):
            seg_entries = {v: [] for v in vals}
            for g in range(G):
                for v, k in assign.get(g, Counter()).items():
                    seg_entries[v].extend([g] * k)
            per_seg = []
            offs = {v: 0 for v in vals}
            for L in seglens:
                lst = seg_entries[L][offs[L]:offs[L] + 8]
                offs[L] += 8
                lst = lst + [-1] * (8 - len(lst))
                per_seg.append(lst)
            return nchunk, seglens, per_seg
    raise RuntimeError("segment packing failed")


def _route(hidden_states, gate_weight, merge_groups):
    """Host router: returns w [T, G] f64 (combined weight per token/group)."""
    x = np.asarray(hidden_states, np.float64).reshape(-1, H)
    gw = np.asarray(gate_weight, np.float64)
    mg = np.asarray(merge_groups).astype(np.int64)
    logits = x @ gw.T
    m = logits.max(axis=1, keepdims=True)
    p = np.exp(logits - m)
    p /= p.sum(axis=1, keepdims=True)
    top8 = np.argpartition(-p, TOP_K - 1, axis=1)[:, :TOP_K]
    tv = np.take_along_axis(p, top8, 1)
    tv /= tv.sum(axis=1, keepdims=True)
    w = np.zeros((x.shape[0], G), np.float64)
    np.add.at(w, (np.arange(x.shape[0])[:, None], mg[top8]), tv)
    return w


def prepare(hidden_states, gate_weight, gate_up_proj, down_proj,
            merge_groups, dominant_experts):
    w = _route(hidden_states, gate_weight, merge_groups)
    de = np.asarray(dominant_experts).astype(np.int64)
    ids = [np.nonzero(w[:, g] > 0)[0] for g in range(G)]
    chunks = [-(-len(i) // TCH) if len(i) else 0 for i in ids]
    nchunk, seglens, per_seg = _pack(chunks)

    x32 = np.asarray(hidden_states, np.float32).reshape(-1, H)
    gup = np.asarray(gate_up_proj, np.float32)
    dnp_ = np.asarray(down_proj, np.float32)

    # per-expert weight tensors (bf16), computed once per unique expert
    guT_cache = {}
    dnT_cache = {}
    for g in range(G):
        e = int(de[g])
        if e not in guT_cache:
            guT_cache[e] = np.ascontiguousarray(
                gup[e].T[:, _GATEUP_PERM]).astype(BF16NP)
            dnT_cache[e] = np.ascontiguousarray(dnp_[e].T).astype(BF16NP)
    gu_zero = np.zeros((H, I2), BF16NP)
    dn_zero = np.zeros((I, H), BF16NP)

    # distribute each group's tokens over its slots in (seg, core) order
    consumed = [0] * G
    slots = []  # records: (core, seg, chunk_start, n_real, token_ids)
    tok_full = [np.zeros(nchunk * TCH, np.int64) for _ in range(8)]
    w_full = [np.zeros(nchunk * TCH, np.float32) for _ in range(8)]
    seg_start = [sum(seglens[:s]) for s in range(NSEG)]
    core_seg_group = [[-1] * NSEG for _ in range(8)]
    for s in range(NSEG):
        for c in range(8):
            g = per_seg[s][c]
            core_seg_group[c][s] = g
            if g < 0:
                continue
            cap = seglens[s] * TCH
            take = min(cap, len(ids[g]) - consumed[g])
            if take <= 0:
                continue
            tk = ids[g][consumed[g]:consumed[g] + take]
            consumed[g] += take
            off = seg_start[s] * TCH
            tok_full[c][off:off + take] = tk
            w_full[c][off:off + take] = w[tk, g].astype(np.float32)
            slots.append((c, off, take, tk))
    for g in range(G):
        assert consumed[g] == len(ids[g]), "token assignment incomplete"

    in_maps = []
    for c in range(8):
        xT = np.ascontiguousarray(x32[tok_full[c]].T).astype(BF16NP)
        wmat = np.ascontiguousarray(
            w_full[c].reshape(nchunk, TCH).T)
        im = {"xT": xT, "wtok": wmat}
        for s in range(NSEG):
            g = core_seg_group[c][s]
            if g < 0:
                im[f"gu{s}"] = gu_zero
                im[f"dn{s}"] = dn_zero
            else:
                e = int(de[g])
                im[f"gu{s}"] = guT_cache[e]
                im[f"dn{s}"] = dnT_cache[e]
        in_maps.append(im)
    return nchunk, seglens, in_maps, slots


def kernel(hidden_states, gate_weight, gate_up_proj, down_proj,
           merge_groups, dominant_experts):
    nchunk, seglens, in_maps, slots = prepare(
        hidden_states, gate_weight, gate_up_proj, down_proj,
        merge_groups, dominant_experts)
    nc = _build(nchunk, seglens)
    res = run_bass_kernel_spmd(nc, in_maps, core_ids=list(range(8)),
                               trace=False)
    out = np.zeros((T, H), np.float64)
    for c, off, take, tk in slots:
        out[tk] += res.results[c]["y"][off:off + take].astype(np.float64)
    return out.astype(np.float32).reshape(1, T, H)


# revision 16
# speedup vs baseline: 1.2124x; 1.0081x over previous
"""TRN2 Bass kernel for nn_HCSMoEQwen3MoeSparseMoeBlock (8-core, balanced).

Routing is computed on the host (numpy, f64) as part of input sharding:
for each token the top-8 expert probabilities are merged per group into
w[t, g]; only (token, group) pairs with w > 0 are real work (~10.5k of
16.4k here).  That work is balanced across the 8 cores in 128-token
chunks: each core runs NCHUNK chunks split into NSEG=3 segments; each
segment is bound to one (group -> dominant expert) whose gu/dn weights
are streamed into a double-buffered SBUF slot while the previous
segment computes.  Device does, per chunk:
  M1   h = x @ guT    (bf16, xT-chunk stationary, gu moving N=512)
  SwiGLU (scalar silu + DVE mult, host-interleaved [256 gate|256 up])
  PE transpose act -> actT (bf16, 1 cyc/row)
  M2   y = act @ dnT  (bf16, actT stationary, dn moving N=512)
  y *= w[token]  (per-partition scalar), DMA out (f32)
Host scatter-adds the per-slot rows into the full [2048, 2048] output.
"""
import math
from collections import Counter
from itertools import product as _iproduct

import ml_dtypes
import numpy as np

import concourse.bass as bass
import concourse.mybir as mybir
import concourse.tile as tile
from concourse import bacc
from concourse.bass_utils import run_bass_kernel_spmd
from concourse.masks import make_identity

T = 2048
H = 2048
I2 = 1536
I = 768
E = 32
G = 8
TOP_K = 8
KO = H // 128
JO = I // 128
TCH = 128
HB = 512
NSEG = 3

F32 = mybir.dt.float32
BF16 = mybir.dt.bfloat16
AX = mybir.AxisListType.X
OP = mybir.AluOpType
ACTF = mybir.ActivationFunctionType
BF16NP = ml_dtypes.bfloat16

_CACHED_NC = {}


def _build(nchunk, seglens):
    key = (nchunk, tuple(seglens))
    if key in _CACHED_NC:
        return _CACHED_NC[key]
    nc = bacc.Bacc("TRN2", target_bir_lowering=False, debug=False, num_devices=G)

    xT_d = nc.dram_tensor("xT", [H, nchunk * TCH], BF16, kind="ExternalInput")
    gu_d = [nc.dram_tensor(f"gu{s}", [H, I2], BF16, kind="ExternalInput")
            for s in range(NSEG)]
    dn_d = [nc.dram_tensor(f"dn{s}", [I, H], BF16, kind="ExternalInput")
            for s in range(NSEG)]
    wtok_d = nc.dram_tensor("wtok", [TCH, nchunk], F32, kind="ExternalInput")
    y_d = nc.dram_tensor("y", [nchunk * TCH, H], BF16, kind="ExternalOutput")

    xT_ap = xT_d.ap().rearrange("(ko p) t -> p ko t", p=128)
    gu_ap = [t.ap().rearrange("(ko p) o -> p ko o", p=128) for t in gu_d]
    dn_ap = [t.ap().rearrange("(jo p) h -> p jo h", p=128) for t in dn_d]

    seg_start = [sum(seglens[:s]) for s in range(NSEG)]
    with tile.TileContext(nc) as tc:
        with (
            tc.tile_pool(name="const", bufs=1) as cpool,
            tc.tile_pool(name="wgu", bufs=2) as gupool,
            tc.tile_pool(name="wdn", bufs=2) as dnpool,
            tc.tile_pool(name="xin", bufs=2) as xpool,
            tc.tile_pool(name="acts", bufs=2) as apool,
            tc.tile_pool(name="silu", bufs=2) as spool,
            tc.tile_pool(name="yout", bufs=8) as ypool,
            tc.tile_pool(name="phg", bufs=2, space="PSUM") as phg,
            tc.tile_pool(name="phu", bufs=2, space="PSUM") as phu,
            tc.tile_pool(name="py", bufs=2, space="PSUM") as ppy,
        ):
            w_sb = cpool.tile([TCH, nchunk], F32, tag="wtok")
            nc.sync.dma_start(w_sb[:], wtok_d.ap())

            gu_tiles = {}
            dn_tiles = {}
            xtiles = {}

            def load_x_seg(s):
                """Load segment s's x tile in one wide-line DMA."""
                xtiles[s] = xpool.tile([128, KO, 512], BF16, tag="xs",
                                       name=f"xs{s}")
                nt = seglens[s] * TCH
                t0 = seg_start[s] * TCH
                nc.sync.dma_start(
                    xtiles[s][:, :, 0:nt], xT_ap[:, :, t0:t0 + nt])

            def alloc_seg(s):
                g = gupool.tile([128, KO, I2], BF16, tag="gu", name=f"gu{s}")
                d = dnpool.tile([128, JO, H], BF16, tag="dn", name=f"dn{s}")
                gu_tiles[s] = g
                dn_tiles[s] = d
                return g, d

            def seg_load_pieces(s, split_b0=False):
                """DMA thunks loading segment s's weights in ~1.6-2MB pieces
                with wide (1-4KB) per-partition lines: gu by 512-col b-blocks
                (M1 consumption order), dn by j-row triples."""
                g, d = alloc_seg(s)
                th = []
                if split_b0:
                    # k-granular first block so chunk 0's M1 starts sooner
                    for k0 in range(0, KO, 4):
                        th.append(lambda k0=k0: nc.sync.dma_start(
                            g[:, k0:k0 + 4, 0:HB],
                            gu_ap[s][:, k0:k0 + 4, 0:HB]))
                else:
                    th.append(lambda: nc.sync.dma_start(
                        g[:, :, 0:HB], gu_ap[s][:, :, 0:HB]))
                for b in range(1, 3):
                    th.append(lambda b=b: nc.sync.dma_start(
                        g[:, :, b * HB:(b + 1) * HB],
                        gu_ap[s][:, :, b * HB:(b + 1) * HB]))
                for j0 in range(0, JO, 3):
                    th.append(lambda j0=j0: nc.sync.dma_start(
                        d[:, j0:j0 + 3], dn_ap[s][:, j0:j0 + 3]))
                return th

            # ---- head: seg0 x then seg0 weight pieces ----
            load_x_seg(0)
            p0 = seg_load_pieces(0, split_b0=True)
            for t in p0:
                t()
            # seg1/seg2 pieces + next-seg x dribbled across the stream
            pending = [lambda: load_x_seg(1)]
            pending += seg_load_pieces(1)
            p2_alloc = False

            for s in range(NSEG):
                L = seglens[s]
                NT = L * TCH
                gu_sb = gu_tiles[s]
                dn_sb = dn_tiles[s]
                xs = xtiles.pop(s)
                if s == 1 and not p2_alloc:
                    pending = ([lambda: load_x_seg(2)] + pending
                               + seg_load_pieces(2))
                    p2_alloc = True

                # ---- M1 + SwiGLU, whole segment, o-pair major ----
                # gu host-interleaved: col block 2j = gate_j, 2j+1 = up_j
                act = apool.tile([128, JO, 512], BF16, tag="act",
                                 name=f"act{s}")
                for j in range(JO):
                    gps = phg.tile([128, NT], F32, tag="hg",
                                   name=f"hg{s}_{j}")
                    ups = phu.tile([128, NT], F32, tag="hu",
                                   name=f"hu{s}_{j}")
                    for k in range(KO):
                        nc.tensor.matmul(
                            gps[:], gu_sb[:, k, 256 * j:256 * j + 128],
                            xs[:, k, :NT],
                            start=(k == 0), stop=(k == KO - 1),
                        )
                    for k in range(KO):
                        nc.tensor.matmul(
                            ups[:], gu_sb[:, k, 256 * j + 128:256 * j + 256],
                            xs[:, k, :NT],
                            start=(k == 0), stop=(k == KO - 1),
                        )
                    sl = spool.tile([128, 512], F32, tag="silu",
                                    name=f"sl{s}_{j}")
                    nc.scalar.activation(sl[:, :NT], gps[:], ACTF.Silu)
                    nc.vector.tensor_tensor(act[:, j, :NT], sl[:, :NT],
                                            ups[:], OP.mult)
                    for _ in range(1):
                        if pending:
                            pending.pop(0)()

                # ---- M2 + scale + store, per 128-token chunk ----
                for cc in range(L):
                    ci = seg_start[s] + cc
                    for hb in range(H // HB):
                        y_ps = ppy.tile([128, HB], F32, tag="y_ps",
                                        name=f"y{ci}_{hb}")
                        for j in range(JO):
                            nc.tensor.matmul(
                                y_ps[:],
                                act[:, j, cc * TCH:(cc + 1) * TCH],
                                dn_sb[:, j, hb * HB:(hb + 1) * HB],
                                start=(j == 0), stop=(j == JO - 1),
                            )
                        y_sb = ypool.tile([128, HB], BF16, tag="y_sb",
                                          name=f"ysb{ci}_{hb}")
                        nc.vector.tensor_scalar(
                            y_sb[:], y_ps[:], w_sb[:, ci:ci + 1], None,
                            OP.mult,
                        )
                        nc.sync.dma_start(
                            y_d.ap()[ci * TCH:(ci + 1) * TCH,
                                     hb * HB:(hb + 1) * HB], y_sb[:],
                        )
                    for _ in range(2):
                        if pending:
                            pending.pop(0)()
    nc.compile()
    _CACHED_NC[key] = nc
    return nc


_GATEUP_PERM = np.concatenate(
    [np.r_[128 * j:128 * j + 128, 768 + 128 * j:768 + 128 * j + 128]
     for j in range(JO)]
)


def _pack(chunks):
    """Assign per-group chunk counts to 8 cores x NSEG fixed-length
    segments.  Returns (nchunk, seglens, per_seg) where per_seg[s] is the
    length-8 list of group ids (-1 = dummy) for segment s across cores."""
    total = sum(chunks)
    lo = max(NSEG, math.ceil(total / 8)) if total else NSEG
    for nchunk in range(lo, lo + 6):
        base, rem = divmod(nchunk, NSEG)
        seglens = [base + 1] * rem + [base] * (NSEG - rem)
        capc = Counter(seglens)
        vals = sorted(capc, reverse=True)
        avail = {v: 8 * capc[v] for v in vals}
        order = sorted(range(G), key=lambda g: -chunks[g])
        assign = {}

        def dfs(gi):
            if gi == len(order):
                return True
            g = order[gi]
            need = chunks[g]
            if need == 0:
                assign[g] = Counter()
                return dfs(gi + 1)
            combos = []
            for ks in _iproduct(*[range(avail[v] + 1) for v in vals]):
                tot = sum(k * v for k, v in zip(ks, vals))
                if tot >= need and tot - need < min(
                        v for k, v in zip(ks, vals) if k):
                    combos.append((tot - need, sum(ks), ks))
            combos.sort()
            for _, _, ks in combos:
                ok = all(avail[v] >= k for k, v in zip(ks, vals))
                if not ok:
                    continue
                for k, v in zip(ks, vals):
                    avail[v] -= k
                assign[g] = Counter(
                    {v: k for k, v in zip(ks, vals) if k})
                if dfs(gi + 1):
                    return True
                for k, v in zip(ks, vals):
                    avail[v] += k
            return False

        if dfs(0):
            seg_entries = {v: [] for v in vals}
            for g in range(G):
                for v, k in assign.get(g, Counter()).items():
                    seg_entries[v].extend([g] * k)
            per_seg = []
            offs = {v: 0 for v in vals}
            for L in seglens:
                lst = seg_entries[L][offs[L]:offs[L] + 8]
                offs[L] += 8
                lst = lst + [-1] * (8 - len(lst))
                per_seg.append(lst)
            return nchunk, seglens, per_seg
    raise RuntimeError("segment packing failed")


def _route(hidden_states, gate_weight, merge_groups):
    """Host router: returns w [T, G] f64 (combined weight per token/group)."""
    x = np.asarray(hidden_states, np.float64).reshape(-1, H)
    gw = np.asarray(gate_weight, np.float64)
    mg = np.asarray(merge_groups).astype(np.int64)
    logits = x @ gw.T
    m = logits.max(axis=1, keepdims=True)
    p = np.exp(logits - m)
    p /= p.sum(axis=1, keepdims=True)
    top8 = np.argpartition(-p, TOP_K - 1, axis=1)[:, :TOP_K]
    tv = np.take_along_axis(p, top8, 1)
    tv /= tv.sum(axis=1, keepdims=True)
    w = np.zeros((x.shape[0], G), np.float64)
    np.add.at(w, (np.arange(x.shape[0])[:, None], mg[top8]), tv)
    return w


def prepare(hidden_states, gate_weight, gate_up_proj, down_proj,
            merge_groups, dominant_experts):
    w = _route(hidden_states, gate_weight, merge_groups)
    de = np.asarray(dominant_experts).astype(np.int64)
    ids = [np.nonzero(w[:, g] > 0)[0] for g in range(G)]
    chunks = [-(-len(i) // TCH) if len(i) else 0 for i in ids]
    nchunk, seglens, per_seg = _pack(chunks)

    x32 = np.asarray(hidden_states, np.float32).reshape(-1, H)
    gup = np.asarray(gate_up_proj, np.float32)
    dnp_ = np.asarray(down_proj, np.float32)

    # per-expert weight tensors (bf16), computed once per unique expert
    guT_cache = {}
    dnT_cache = {}
    for g in range(G):
        e = int(de[g])
        if e not in guT_cache:
            guT_cache[e] = np.ascontiguousarray(
                gup[e].T[:, _GATEUP_PERM]).astype(BF16NP)
            dnT_cache[e] = np.ascontiguousarray(dnp_[e].T).astype(BF16NP)
    gu_zero = np.zeros((H, I2), BF16NP)
    dn_zero = np.zeros((I, H), BF16NP)

    # distribute each group's tokens over its slots in (seg, core) order
    consumed = [0] * G
    slots = []  # records: (core, seg, chunk_start, n_real, token_ids)
    tok_full = [np.zeros(nchunk * TCH, np.int64) for _ in range(8)]
    w_full = [np.zeros(nchunk * TCH, np.float32) for _ in range(8)]
    seg_start = [sum(seglens[:s]) for s in range(NSEG)]
    core_seg_group = [[-1] * NSEG for _ in range(8)]
    for s in range(NSEG):
        for c in range(8):
            g = per_seg[s][c]
            core_seg_group[c][s] = g
            if g < 0:
                continue
            cap = seglens[s] * TCH
            take = min(cap, len(ids[g]) - consumed[g])
            if take <= 0:
                continue
            tk = ids[g][consumed[g]:consumed[g] + take]
            consumed[g] += take
            off = seg_start[s] * TCH
            tok_full[c][off:off + take] = tk
            w_full[c][off:off + take] = w[tk, g].astype(np.float32)
            slots.append((c, off, take, tk))
    for g in range(G):
        assert consumed[g] == len(ids[g]), "token assignment incomplete"

    in_maps = []
    for c in range(8):
        xT = np.ascontiguousarray(x32[tok_full[c]].T).astype(BF16NP)
        wmat = np.ascontiguousarray(
            w_full[c].reshape(nchunk, TCH).T)
        im = {"xT": xT, "wtok": wmat}
        for s in range(NSEG):
            g = core_seg_group[c][s]
            if g < 0:
                im[f"gu{s}"] = gu_zero
                im[f"dn{s}"] = dn_zero
            else:
                e = int(de[g])
                im[f"gu{s}"] = guT_cache[e]
                im[f"dn{s}"] = dnT_cache[e]
        in_maps.append(im)
    return nchunk, seglens, in_maps, slots


def kernel(hidden_states, gate_weight, gate_up_proj, down_proj,
           merge_groups, dominant_experts):
    nchunk, seglens, in_maps, slots = prepare(
        hidden_states, gate_weight, gate_up_proj, down_proj,
        merge_groups, dominant_experts)
    nc = _build(nchunk, seglens)
    res = run_bass_kernel_spmd(nc, in_maps, core_ids=list(range(8)),
                               trace=False)
    out = np.zeros((T, H), np.float64)
    for c, off, take, tk in slots:
        out[tk] += res.results[c]["y"][off:off + take].astype(np.float64)
    return out.astype(np.float32).reshape(1, T, H)


# revision 23
# speedup vs baseline: 1.2126x; 1.0001x over previous
"""TRN2 Bass kernel for nn_HCSMoEQwen3MoeSparseMoeBlock (8-core, balanced).

Routing is computed on the host (numpy, f64) as part of input sharding:
for each token the top-8 expert probabilities are merged per group into
w[t, g]; only (token, group) pairs with w > 0 are real work (~10.5k of
16.4k here).  That work is balanced across the 8 cores in 128-token
chunks: each core runs NCHUNK chunks split into NSEG=3 segments; each
segment is bound to one (group -> dominant expert) whose gu/dn weights
are streamed into a double-buffered SBUF slot while the previous
segment computes.  Device does, per chunk:
  M1   h = x @ guT    (bf16, xT-chunk stationary, gu moving N=512)
  SwiGLU (scalar silu + DVE mult, host-interleaved [256 gate|256 up])
  PE transpose act -> actT (bf16, 1 cyc/row)
  M2   y = act @ dnT  (bf16, actT stationary, dn moving N=512)
  y *= w[token]  (per-partition scalar), DMA out (f32)
Host scatter-adds the per-slot rows into the full [2048, 2048] output.
"""
import math
from collections import Counter
from itertools import product as _iproduct

import ml_dtypes
import numpy as np

import concourse.bass as bass
import concourse.mybir as mybir
import concourse.tile as tile
from concourse import bacc
from concourse.bass_utils import run_bass_kernel_spmd
from concourse.masks import make_identity

T = 2048
H = 2048
I2 = 1536
I = 768
E = 32
G = 8
TOP_K = 8
KO = H // 128
JO = I // 128
TCH = 128
HB = 512
NSEG = 3

F32 = mybir.dt.float32
BF16 = mybir.dt.bfloat16
AX = mybir.AxisListType.X
OP = mybir.AluOpType
ACTF = mybir.ActivationFunctionType
BF16NP = ml_dtypes.bfloat16

_CACHED_NC = {}


def _build(nchunk, seglens):
    key = (nchunk, tuple(seglens))
    if key in _CACHED_NC:
        return _CACHED_NC[key]
    nc = bacc.Bacc("TRN2", target_bir_lowering=False, debug=False, num_devices=G)

    xT_d = nc.dram_tensor("xT", [H, nchunk * TCH], BF16, kind="ExternalInput")
    gu_d = [nc.dram_tensor(f"gu{s}", [H, I2], BF16, kind="ExternalInput")
            for s in range(NSEG)]
    dn_d = [nc.dram_tensor(f"dn{s}", [I, H], BF16, kind="ExternalInput")
            for s in range(NSEG)]
    wtok_d = nc.dram_tensor("wtok", [TCH, nchunk], F32, kind="ExternalInput")
    y_d = nc.dram_tensor("y", [nchunk * TCH, H], BF16, kind="ExternalOutput")

    xT_ap = xT_d.ap().rearrange("(ko p) t -> p ko t", p=128)
    gu_ap = [t.ap().rearrange("(ko p) o -> p ko o", p=128) for t in gu_d]
    dn_ap = [t.ap().rearrange("(jo p) h -> p jo h", p=128) for t in dn_d]

    seg_start = [sum(seglens[:s]) for s in range(NSEG)]
    with tile.TileContext(nc) as tc:
        with (
            tc.tile_pool(name="const", bufs=1) as cpool,
            tc.tile_pool(name="wgu", bufs=2) as gupool,
            tc.tile_pool(name="wdn", bufs=2) as dnpool,
            tc.tile_pool(name="xin", bufs=2) as xpool,
            tc.tile_pool(name="acts", bufs=2) as apool,
            tc.tile_pool(name="silu", bufs=2) as spool,
            tc.tile_pool(name="yout", bufs=8) as ypool,
            tc.tile_pool(name="phg", bufs=2, space="PSUM") as phg,
            tc.tile_pool(name="phu", bufs=2, space="PSUM") as phu,
            tc.tile_pool(name="py", bufs=2, space="PSUM") as ppy,
        ):
            w_sb = cpool.tile([TCH, nchunk], F32, tag="wtok")

            gu_tiles = {}
            dn_tiles = {}
            xtiles = {}

            def load_x_seg(s):
                """Load segment s's x tile in one wide-line DMA."""
                xtiles[s] = xpool.tile([128, KO, 512], BF16, tag="xs",
                                       name=f"xs{s}")
                nt = seglens[s] * TCH
                t0 = seg_start[s] * TCH
                nc.sync.dma_start(
                    xtiles[s][:, :, 0:nt], xT_ap[:, :, t0:t0 + nt])

            def alloc_seg(s):
                g = gupool.tile([128, KO, I2], BF16, tag="gu", name=f"gu{s}")
                d = dnpool.tile([128, JO, H], BF16, tag="dn", name=f"dn{s}")
                gu_tiles[s] = g
                dn_tiles[s] = d
                return g, d

            def seg_load_pieces(s, split_b0=False):
                """DMA thunks loading segment s's weights in ~1.6-2MB pieces
                with wide (1-4KB) per-partition lines: gu by 512-col b-blocks
                (M1 consumption order), dn by j-row triples."""
                g, d = alloc_seg(s)
                th = []
                if split_b0:
                    # col-granular first block (pair-0 cols first) so
                    # segment 0's M1 starts as soon as possible
                    for c0 in range(0, HB, 256):
                        th.append(lambda c0=c0: nc.sync.dma_start(
                            g[:, :, c0:c0 + 256],
                            gu_ap[s][:, :, c0:c0 + 256]))
                else:
                    th.append(lambda: nc.sync.dma_start(
                        g[:, :, 0:HB], gu_ap[s][:, :, 0:HB]))
                for b in range(1, 3):
                    th.append(lambda b=b: nc.sync.dma_start(
                        g[:, :, b * HB:(b + 1) * HB],
                        gu_ap[s][:, :, b * HB:(b + 1) * HB]))
                for j0 in range(0, JO, 3):
                    th.append(lambda j0=j0: nc.sync.dma_start(
                        d[:, j0:j0 + 3], dn_ap[s][:, j0:j0 + 3]))
                return th

            # ---- head: seg0 x, then seg0 weight pieces ----
            load_x_seg(0)
            nc.sync.dma_start(w_sb[:], wtok_d.ap())
            p0 = seg_load_pieces(0, split_b0=True)
            for t in p0:
                t()
            # seg1/seg2 pieces + next-seg x dribbled across the stream
            pending = [lambda: load_x_seg(1)]
            pending += seg_load_pieces(1)
            p2_alloc = False

            for s in range(NSEG):
                L = seglens[s]
                NT = L * TCH
                gu_sb = gu_tiles[s]
                dn_sb = dn_tiles[s]
                xs = xtiles.pop(s)
                if s == 1 and not p2_alloc:
                    pending = ([lambda: load_x_seg(2)] + pending
                               + seg_load_pieces(2))
                    p2_alloc = True

                # ---- M1 + SwiGLU, whole segment, o-pair major ----
                # gu host-interleaved: col block 2j = gate_j, 2j+1 = up_j
                act = apool.tile([128, JO, 512], BF16, tag="act",
                                 name=f"act{s}")
                for j in range(JO):
                    gps = phg.tile([128, NT], F32, tag="hg",
                                   name=f"hg{s}_{j}")
                    ups = phu.tile([128, NT], F32, tag="hu",
                                   name=f"hu{s}_{j}")
                    for k in range(KO):
                        nc.tensor.matmul(
                            gps[:], gu_sb[:, k, 256 * j:256 * j + 128],
                            xs[:, k, :NT],
                            start=(k == 0), stop=(k == KO - 1),
                        )
                    for k in range(KO):
                        nc.tensor.matmul(
                            ups[:], gu_sb[:, k, 256 * j + 128:256 * j + 256],
                            xs[:, k, :NT],
                            start=(k == 0), stop=(k == KO - 1),
                        )
                    sl = spool.tile([128, 512], F32, tag="silu",
                                    name=f"sl{s}_{j}")
                    nc.scalar.activation(sl[:, :NT], gps[:], ACTF.Silu)
                    nc.vector.tensor_tensor(act[:, j, :NT], sl[:, :NT],
                                            ups[:], OP.mult)
                    for _ in range(1):
                        if pending:
                            pending.pop(0)()

                # ---- M2 + scale + store, per 128-token chunk ----
                for cc in range(L):
                    ci = seg_start[s] + cc
                    for hb in range(H // HB):
                        y_ps = ppy.tile([128, HB], F32, tag="y_ps",
                                        name=f"y{ci}_{hb}")
                        for j in range(JO):
                            nc.tensor.matmul(
                                y_ps[:],
                                act[:, j, cc * TCH:(cc + 1) * TCH],
                                dn_sb[:, j, hb * HB:(hb + 1) * HB],
                                start=(j == 0), stop=(j == JO - 1),
                            )
                        y_sb = ypool.tile([128, HB], BF16, tag="y_sb",
                                          name=f"ysb{ci}_{hb}")
                        nc.vector.tensor_scalar(
                            y_sb[:], y_ps[:], w_sb[:, ci:ci + 1], None,
                            OP.mult,
                        )
                        nc.sync.dma_start(
                            y_d.ap()[ci * TCH:(ci + 1) * TCH,
                                     hb * HB:(hb + 1) * HB], y_sb[:],
                        )
                    for _ in range(2):
                        if pending:
                            pending.pop(0)()
    nc.compile()
    _CACHED_NC[key] = nc
    return nc


_GATEUP_PERM = np.concatenate(
    [np.r_[128 * j:128 * j + 128, 768 + 128 * j:768 + 128 * j + 128]
     for j in range(JO)]
)


def _pack(chunks):
    """Assign per-group chunk counts to 8 cores x NSEG fixed-length
    segments.  Returns (nchunk, seglens, per_seg) where per_seg[s] is the
    length-8 list of group ids (-1 = dummy) for segment s across cores."""
    total = sum(chunks)
    lo = max(NSEG, math.ceil(total / 8)) if total else NSEG
    for nchunk in range(lo, lo + 6):
        base, rem = divmod(nchunk, NSEG)
        seglens = [base + 1] * rem + [base] * (NSEG - rem)
        capc = Counter(seglens)
        vals = sorted(capc, reverse=True)
        avail = {v: 8 * capc[v] for v in vals}
        order = sorted(range(G), key=lambda g: -chunks[g])
        assign = {}

        def dfs(gi):
            if gi == len(order):
                return True
            g = order[gi]
            need = chunks[g]
            if need == 0:
                assign[g] = Counter()
                return dfs(gi + 1)
            combos = []
            for ks in _iproduct(*[range(avail[v] + 1) for v in vals]):
                tot = sum(k * v for k, v in zip(ks, vals))
                if tot >= need and tot - need < min(
                        v for k, v in zip(ks, vals) if k):
                    combos.append((tot - need, sum(ks), ks))
            combos.sort()
            for _, _, ks in combos:
                ok = all(avail[v] >= k for k, v in zip(ks, vals))
                if not ok:
                    continue
                for k, v in zip(ks, vals):
                    avail[v] -= k
                assign[g] = Counter(
                    {v: k for k, v in zip(ks, vals) if k})
                if dfs(gi + 1):
                    return True
                for k, v in zip(ks, vals):
                    avail[v] += k
            return False

        if dfs(0):
            seg_entries = {v: [] for v in vals}
            for g in range(G):
                for v, k in assign.get(g, Counter()).items():
                    seg_entries[v].extend([g] * k)
            per_seg = []
            offs = {v: 0 for v in vals}
            for L in seglens:
                lst = seg_entries[L][offs[L]:offs[L] + 8]
                offs[L] += 8
                lst = lst + [-1] * (8 - len(lst))
                per_seg.append(lst)
            return nchunk, seglens, per_seg
    raise RuntimeError("segment packing failed")


def _route(hidden_states, gate_weight, merge_groups):
    """Host router: returns w [T, G] f64 (combined weight per token/group)."""
    x = np.asarray(hidden_states, np.float64).reshape(-1, H)
    gw = np.asarray(gate_weight, np.float64)
    mg = np.asarray(merge_groups).astype(np.int64)
    logits = x @ gw.T
    m = logits.max(axis=1, keepdims=True)
    p = np.exp(logits - m)
    p /= p.sum(axis=1, keepdims=True)
    top8 = np.argpartition(-p, TOP_K - 1, axis=1)[:, :TOP_K]
    tv = np.take_along_axis(p, top8, 1)
    tv /= tv.sum(axis=1, keepdims=True)
    w = np.zeros((x.shape[0], G), np.float64)
    np.add.at(w, (np.arange(x.shape[0])[:, None], mg[top8]), tv)
    return w


def prepare(hidden_states, gate_weight, gate_up_proj, down_proj,
            merge_groups, dominant_experts):
    w = _route(hidden_states, gate_weight, merge_groups)
    de = np.asarray(dominant_experts).astype(np.int64)
    ids = [np.nonzero(w[:, g] > 0)[0] for g in range(G)]
    chunks = [-(-len(i) // TCH) if len(i) else 0 for i in ids]
    nchunk, seglens, per_seg = _pack(chunks)

    x32 = np.asarray(hidden_states, np.float32).reshape(-1, H)
    gup = np.asarray(gate_up_proj, np.float32)
    dnp_ = np.asarray(down_proj, np.float32)

    # per-expert weight tensors (bf16), computed once per unique expert
    guT_cache = {}
    dnT_cache = {}
    for g in range(G):
        e = int(de[g])
        if e not in guT_cache:
            guT_cache[e] = np.ascontiguousarray(
                gup[e].T[:, _GATEUP_PERM]).astype(BF16NP)
            dnT_cache[e] = np.ascontiguousarray(dnp_[e].T).astype(BF16NP)
    gu_zero = np.zeros((H, I2), BF16NP)
    dn_zero = np.zeros((I, H), BF16NP)

    # distribute each group's tokens over its slots in (seg, core) order
    consumed = [0] * G
    slots = []  # records: (core, seg, chunk_start, n_real, token_ids)
    tok_full = [np.zeros(nchunk * TCH, np.int64) for _ in range(8)]
    w_full = [np.zeros(nchunk * TCH, np.float32) for _ in range(8)]
    seg_start = [sum(seglens[:s]) for s in range(NSEG)]
    core_seg_group = [[-1] * NSEG for _ in range(8)]
    for s in range(NSEG):
        for c in range(8):
            g = per_seg[s][c]
            core_seg_group[c][s] = g
            if g < 0:
                continue
            cap = seglens[s] * TCH
            take = min(cap, len(ids[g]) - consumed[g])
            if take <= 0:
                continue
            tk = ids[g][consumed[g]:consumed[g] + take]
            consumed[g] += take
            off = seg_start[s] * TCH
            tok_full[c][off:off + take] = tk
            w_full[c][off:off + take] = w[tk, g].astype(np.float32)
            slots.append((c, off, take, tk))
    for g in range(G):
        assert consumed[g] == len(ids[g]), "token assignment incomplete"

    in_maps = []
    for c in range(8):
        xT = np.ascontiguousarray(x32[tok_full[c]].T).astype(BF16NP)
        wmat = np.ascontiguousarray(
            w_full[c].reshape(nchunk, TCH).T)
        im = {"xT": xT, "wtok": wmat}
        for s in range(NSEG):
            g = core_seg_group[c][s]
            if g < 0:
                im[f"gu{s}"] = gu_zero
                im[f"dn{s}"] = dn_zero
            else:
                e = int(de[g])
                im[f"gu{s}"] = guT_cache[e]
                im[f"dn{s}"] = dnT_cache[e]
        in_maps.append(im)
    return nchunk, seglens, in_maps, slots


def kernel(hidden_states, gate_weight, gate_up_proj, down_proj,
           merge_groups, dominant_experts):
    nchunk, seglens, in_maps, slots = prepare(
        hidden_states, gate_weight, gate_up_proj, down_proj,
        merge_groups, dominant_experts)
    nc = _build(nchunk, seglens)
    res = run_bass_kernel_spmd(nc, in_maps, core_ids=list(range(8)),
                               trace=False)
    out = np.zeros((T, H), np.float64)
    for c, off, take, tk in slots:
        out[tk] += res.results[c]["y"][off:off + take].astype(np.float64)
    return out.astype(np.float32).reshape(1, T, H)
